# revision 7
# baseline (speedup 1.0000x reference)
"""DANet3D dual-attention kernel for Trainium2 (8 NeuronCores, Bass/Tile).

Sharding: x -> proj p [2, 64, 8000]; 8 cores = 2 batches x 4 query-blocks
of 2000 positions.  Each core receives the full batch projection (keys /
values / channel attention) plus its own query block and computes its
[64, 2000] slice of the output.

Position attention (per batch):
  E[n,m] = (Wq p_n + bq).(Wk p_m + bk)
         = p_n^T M p_m + w.p_m + row-constants,   M = Wq^T Wk, w = Wk^T bq
  softmax_m is invariant to row-constants, so with kp = M p:
  attn = rowsoftmax( exp(kp_m . p_n + w.p_m) ).
  Flash loop over 63 key tiles in E^T layout [keys x queries]:
  F = exp(kp_tile^T p_q + w.p_m)  -- the per-key bias rides the ACT
  instruction's free per-partition bias slot (no separate g scaling).
  U[c,n] += vt[m,c] F[m,n]  with vt = [gamma_p*(Wv p + bv)^T | 1]: the
  ones column makes U[64] the softmax denominator.

Performance structure (from HW traces): the PE only reaches 2.4 GHz after
a ~3.4us continuous-busy window and re-throttles to 1.2 GHz after a
~3.4us idle window, so the kernel (a) fires a dense warm-up burst of
matmuls immediately, (b) software-pipelines the kp/vt/Gram projection
matmuls INTO the flash loop so the PE never idles long, and (c) uses one
flat PSUM layout (two F tags + two U halves = 8 banks) where every
transient borrows an F tag slot -- no pool-boundary serialization.
QK pairs are emitted as 2-way row-packed tile_position pairs (rows 0:64 /
64:128), which the hardware can run concurrently.

Channel attention: the p p^T Gram accumulates in per-iteration groups
(flushed to SBUF by DVE) inside the loop; its softmax/transpose/outc2 run
in the epilogue, ordered after the last flash exp so the single ACT
engine's FIFO is never blocked.  ac2 = gamma_c*attn_c^T + 2I folds into
one fp32 matmul  outc2 = ac2^T p_q = gamma_c*out_c + 2x (exact when
gamma=0).  Final: out = U[0:64] * (1/U[64]) + outc2.
"""

from contextlib import ExitStack

import ml_dtypes
import numpy as np

import concourse.bass as bass
import concourse.mybir as mybir
import concourse.tile as tile
from concourse import bacc
from concourse.bass import ds, ts
from concourse.bass_utils import run_bass_kernel_spmd
from concourse.masks import make_identity
from concourse.tile import add_dep_helper

F32 = mybir.dt.float32
BF16 = mybir.dt.bfloat16
AF = mybir.ActivationFunctionType
ALU = mybir.AluOpType
AX = mybir.AxisListType

B, C, D, H, W = 2, 64, 20, 20, 20
N = D * H * W            # 8000
MT = 128                 # key (m) tile size
NRT = 63                 # real m tiles (tile 63 is pure padding, skipped)
NPAD = 8192              # padded key range (64 tiles)
HALF = NPAD // 2         # 4096 (m-tile pair split)
NPAIR = 32               # pair iterations (A=i, B=32+i)
NQ = N // 4              # 2000 queries per core
NH = NQ // 2             # 1000 = one exp-op worth of queries
KCH = 512                # kp projection chunk
CH = (512, 488)          # query sub-chunks, each within one PSUM bank
LA = 4                   # software-pipeline lookahead (m tiles)
NCORES = 8


def build_danet(ctx, tc, io):
    nc = tc.nc
    xbb, xq, xqb2 = io["xbb"], io["xq"], io["xqb2"]
    mpT, wvx, gc, gp, eye2, out_d = (io["mpT"], io["wvx"], io["gc"],
                                     io["gp"], io["eye2"], io["out"])

    persist = ctx.enter_context(tc.tile_pool(name="persist", bufs=1))
    fs_pool = ctx.enter_context(tc.tile_pool(name="fs", bufs=4))
    up = ctx.enter_context(tc.tile_pool(name="ps_u", bufs=1, space="PSUM"))
    fp = ctx.enter_context(tc.tile_pool(name="ps_f", bufs=1, space="PSUM"))

    pab = persist.tile([65, NPAD], BF16)      # bf16 proj + ones row (host)
    paq = persist.tile([64, NQ], F32)         # query block fp32 (outc2)
    paqb2 = persist.tile([128, NQ], BF16)     # query block bf16, duplicated
    kp2 = persist.tile([128, HALF], BF16)     # M@p packed halves
    vraw = persist.tile([128, NRT, 65], F32)  # unscaled [vT+bv | w.p]
    vt = persist.tile([128, NRT, 65], BF16)   # [gamma_p*vT | 1]
    pt = persist.tile([128, NRT, 64], BF16)   # projT tiles (channel attn)
    mpT_s = persist.tile([64, 64], BF16)
    wvx_s = persist.tile([65, 129], BF16)
    gc_s = persist.tile([64, 1], F32)
    gp_s = persist.tile([128, 1], F32)
    eye2_s = persist.tile([64, 64], F32)
    id64 = persist.tile([64, 64], F32)
    ones_s = persist.tile([1, 64], F32)
    ec_acc = persist.tile([64, 64], F32)
    ee = persist.tile([64, 64], F32)
    ac2 = persist.tile([64, 64], F32)
    mx = persist.tile([64, 1], F32)
    sc = persist.tile([64, 1], F32)
    rc = persist.tile([64, 1], F32)
    rcg = persist.tile([64, 1], F32)
    oc_sb = persist.tile([64, NQ], F32)       # gamma_c*out_c + 2x
    rec = persist.tile([1, NQ], F32)
    bc_sb = persist.tile([64, NQ], F32)
    out_sb = persist.tile([64, NQ], F32)

    # ---- input DMAs (ordered so bootstrap consumers land first) ----
    nc.sync.dma_start(out=mpT_s, in_=mpT)
    nc.sync.dma_start(out=wvx_s, in_=wvx)
    nc.sync.dma_start(out=paqb2, in_=xqb2)
    nc.sync.dma_start(out=gc_s, in_=gc)
    nc.sync.dma_start(out=gp_s, in_=gp)
    nc.sync.dma_start(out=eye2_s, in_=eye2)
    nc.sync.dma_start(out=paq, in_=xq)
    xw = NPAD // 8
    for i in (0, 4, 1, 5, 2, 6, 3, 7):
        nc.sync.dma_start(out=pab[:, ts(i, xw)], in_=xbb[:, ts(i, xw)])
    make_identity(nc, id64)
    nc.vector.memset(ones_s, 1.0)
    nc.vector.memset(ec_acc, 0.0)
    nc.vector.memset(vt[:, :, 64:65], 1.0)    # ones column (pad fixed later)

    tag_n = [0]

    def tagf():
        tag_n[0] += 1
        return "f_a" if tag_n[0] % 2 else "f_b"

    # ---- PE warm-up burst: ~5us of dense matmuls (HAM -> K=8/8) ----
    warm = fp.tile([128, 1024], F32, name="warm", tag=tagf())
    for r in range(12):
        nc.tensor.matmul(warm[0:64, 0:512], paqb2[0:64, 0:64],
                         paqb2[0:64, 0:512], start=True, stop=True,
                         skip_group_check=True)

    def emit_kp(c):
        """kp2 chunk c (0..15): cols c%8*512 of half c//8."""
        half = c // 8
        sl = slice(half * 64, half * 64 + 64)
        kp_ps = fp.tile([128, KCH], F32, name=f"kp{c}", tag=tagf())
        nc.tensor.matmul(kp_ps[sl, :], mpT_s,
                         pab[0:64, ds(half * HALF + (c % 8) * KCH, KCH)],
                         start=True, stop=True,
                         tile_position=(0, half * 64))
        nc.vector.tensor_copy(out=kp2[sl, ts(c % 8, KCH)], in_=kp_ps[sl, :])

    def emit_vt(t):
        vt_ps = fp.tile([128, 129], F32, name=f"vt{t}", tag=tagf())
        nc.tensor.matmul(vt_ps, pab[:, ts(t, MT)], wvx_s,
                         start=True, stop=True)
        nc.vector.tensor_copy(out=vraw[:, t, :], in_=vt_ps[:, 0:65])
        nc.vector.tensor_copy(out=pt[:, t, :], in_=vt_ps[:, 65:129])
        nc.vector.tensor_scalar_mul(out=vt[:, t, 0:64],
                                    in0=vraw[:, t, 0:64], scalar1=gp_s)
        if t == NRT - 1:  # zero pad keys m in [8000, 8064)
            nc.vector.memset(vt[64:128, t, :], 0.0)

    def emit_gram(tiles):
        g_ps = fp.tile([64, 64], F32, name=f"g{tiles[0]}", tag=tagf())
        for k, t in enumerate(tiles):
            nc.tensor.matmul(g_ps, pt[:, t, :], pt[:, t, :],
                             start=(k == 0), stop=(k == len(tiles) - 1))
        nc.vector.tensor_tensor(out=ec_acc, in0=ec_acc, in1=g_ps, op=ALU.add)

    # ---- bootstrap the pipeline ----
    emit_kp(0)
    emit_kp(8)
    for t in (0, 1, 2, 3, 32, 33, 34, 35):
        emit_vt(t)

    # ---- main flash loop (prologue pipelined in) ----
    u_ps = [up.tile([65, 1024], F32, name=f"u{h}", tag=f"u{h}")
            for h in range(2)]
    last_exp = None
    for i in range(NPAIR):
        tb = NPAIR + i
        has_b = tb < NRT
        # pipelined projections for future iterations
        if i % 4 == 0 and i // 4 + 1 <= 7:
            emit_kp(i // 4 + 1)
            emit_kp(8 + i // 4 + 1)
        if i + LA <= NPAIR - 1:
            emit_vt(i + LA)
        if 32 + i + LA <= NRT - 1:
            emit_vt(32 + i + LA)
        if i >= 2:
            g_tiles = [i - 2]
            if 30 + i <= NRT - 1:
                g_tiles.append(30 + i)
            emit_gram(g_tiles)

        for h in range(2):
            f_a = fp.tile([128, 1024], F32, name="f_a", tag="f_a")
            f_b = (fp.tile([128, 1024], F32, name="f_b", tag="f_b")
                   if has_b else None)
            for off, w_ in zip((0, 512), CH):
                nc.tensor.matmul(f_a[:, ds(off, w_)], kp2[0:64, ts(i, MT)],
                                 paqb2[0:64, ds(h * NH + off, w_)],
                                 start=True, stop=True, tile_position=(0, 0))
                if has_b:
                    nc.tensor.matmul(f_b[:, ds(off, w_)],
                                     kp2[64:128, ts(i, MT)],
                                     paqb2[64:128, ds(h * NH + off, w_)],
                                     start=True, stop=True,
                                     tile_position=(64, 0))
            fsb_a = fs_pool.tile([128, 1000], BF16, name="fsb", tag="fsb")
            last_exp = nc.scalar.activation(out=fsb_a, in_=f_a[:, 0:1000],
                                            func=AF.Exp,
                                            bias=vraw[:, i, 64:65])
            if has_b:
                fsb_b = fs_pool.tile([128, 1000], BF16, name="fsb",
                                     tag="fsb")
                last_exp = nc.scalar.activation(out=fsb_b,
                                                in_=f_b[:, 0:1000],
                                                func=AF.Exp,
                                                bias=vraw[:, tb, 64:65])
            for off, w_ in zip((0, 512), CH):
                nc.tensor.matmul(u_ps[h][:, ds(off, w_)], vt[:, i, :],
                                 fsb_a[:, ds(off, w_)],
                                 start=(i == 0), stop=(i == NPAIR - 1))
                if has_b:
                    nc.tensor.matmul(u_ps[h][:, ds(off, w_)], vt[:, tb, :],
                                     fsb_b[:, ds(off, w_)],
                                     start=False, stop=False)
    emit_gram([30, 31, 62])  # leftover Gram tiles

    # ---- epilogue: channel attention softmax + normalize/combine ----
    nc.vector.tensor_reduce(out=mx, in_=ec_acc, axis=AX.X, op=ALU.max,
                            negate=True)
    ee_inst = nc.scalar.activation(out=ee, in_=ec_acc, func=AF.Exp, bias=mx)
    if last_exp is not None:  # keep ACT FIFO clear for the flash exps
        add_dep_helper(ee_inst.ins, last_exp.ins, sync=False,
                       reason="channel softmax after flash exps")
    nc.vector.tensor_reduce(out=sc, in_=ee, axis=AX.X, op=ALU.add)
    nc.vector.reciprocal(out=rc, in_=sc)
    nc.vector.tensor_mul(out=rcg, in0=rc, in1=gc_s)
    nc.vector.tensor_scalar_mul(out=ee, in0=ee, scalar1=rcg)
    at_ps = fp.tile([64, 64], F32, name="at_ps", tag=tagf())
    nc.tensor.transpose(at_ps, ee, id64)
    nc.vector.tensor_add(out=ac2, in0=at_ps, in1=eye2_s)
    for j in range(4):  # outc2 = gamma_c*out_c + 2x (fp32: exact 2x)
        oc_ps = fp.tile([64, 500], F32, name=f"oc{j}", tag=tagf())
        nc.tensor.matmul(oc_ps, ac2, paq[:, ts(j, 500)],
                         start=True, stop=True)
        nc.vector.tensor_copy(out=oc_sb[:, ts(j, 500)], in_=oc_ps)

    for h in range(2):
        uh = u_ps[h]
        nc.vector.reciprocal(out=rec[:, ds(h * NH, NH)],
                             in_=uh[64:65, 0:1000])
        bc_ps = fp.tile([64, 1024], F32, name=f"bc{h}", tag=tagf())
        for off, w_ in zip((0, 512), CH):
            nc.tensor.matmul(bc_ps[:, ds(off, w_)], ones_s,
                             rec[:, ds(h * NH + off, w_)],
                             start=True, stop=True)
        nc.vector.tensor_copy(out=bc_sb[:, ds(h * NH, NH)],
                              in_=bc_ps[:, 0:1000])
        o_h = out_sb[:, ds(h * NH, NH)]
        nc.vector.tensor_mul(out=o_h, in0=uh[0:64, 0:1000],
                             in1=bc_sb[:, ds(h * NH, NH)])
        nc.vector.tensor_add(out=o_h, in0=o_h, in1=oc_sb[:, ds(h * NH, NH)])
    nc.sync.dma_start(out=out_d, in_=out_sb)


def _mk_io(nc):
    io = {}
    io["xbb"] = nc.dram_tensor("xbb", [65, NPAD], BF16,
                               kind="ExternalInput").ap()
    io["xq"] = nc.dram_tensor("xq", [64, NQ], F32, kind="ExternalInput").ap()
    io["xqb2"] = nc.dram_tensor("xqb2", [128, NQ], BF16,
                                kind="ExternalInput").ap()
    io["mpT"] = nc.dram_tensor("mpT", [64, 64], BF16,
                               kind="ExternalInput").ap()
    io["wvx"] = nc.dram_tensor("wvx", [65, 129], BF16,
                               kind="ExternalInput").ap()
    io["gc"] = nc.dram_tensor("gc", [64, 1], F32, kind="ExternalInput").ap()
    io["gp"] = nc.dram_tensor("gp", [128, 1], F32, kind="ExternalInput").ap()
    io["eye2"] = nc.dram_tensor("eye2", [64, 64], F32,
                                kind="ExternalInput").ap()
    io["out"] = nc.dram_tensor("out", [64, NQ], F32,
                               kind="ExternalOutput").ap()
    return io


_CACHE = {}


def build_program():
    if "nc" not in _CACHE:
        nc = bacc.Bacc("TRN2", target_bir_lowering=False, debug=False,
                       num_devices=NCORES)
        io = _mk_io(nc)
        with tile.TileContext(nc) as tc, ExitStack() as ctx:
            build_danet(ctx, tc, io)
        nc.compile()
        _CACHE["nc"] = nc
    return _CACHE["nc"]


def make_in_maps(x, Wq, bq, Wk, bk, Wv, bv, gamma_c, gamma_p):
    f = np.float32
    bf = ml_dtypes.bfloat16
    proj = np.asarray(x, f).reshape(B, C, N)
    Wq, bq, Wk, bk = (np.asarray(a, f) for a in (Wq, bq, Wk, bk))
    Wv, bv = np.asarray(Wv, f), np.asarray(bv, f)
    gamma_c = float(np.asarray(gamma_c).reshape(-1)[0])
    gamma_p = float(np.asarray(gamma_p).reshape(-1)[0])

    mpT = (Wq.T @ Wk).T.astype(bf)       # lhsT for kp = M @ p
    w = (Wk.T @ bq).astype(f)            # per-key bias (rides exp's bias)
    wvx = np.zeros((65, 129), f)
    wvx[0:64, 0:64] = Wv.T
    wvx[64, 0:64] = bv
    wvx[0:64, 64] = w
    wvx[0:64, 65:129] = np.eye(64, dtype=f)
    wvx = wvx.astype(bf)
    gc = np.full((64, 1), gamma_c, f)
    gp = np.full((128, 1), gamma_p, f)
    eye2 = (2.0 * np.eye(64)).astype(f)

    in_maps = []
    for core in range(NCORES):
        b, qb = divmod(core, 4)
        xbuf = np.zeros((65, NPAD), f)
        xbuf[0:64, 0:N] = proj[b]
        xbuf[64, :] = 1.0
        xqf = np.ascontiguousarray(proj[b][:, qb * NQ:(qb + 1) * NQ])
        xqb2 = np.broadcast_to(xqf.astype(bf), (2, 64, NQ)).reshape(128, NQ)
        in_maps.append({"xbb": xbuf.astype(bf), "xq": xqf,
                        "xqb2": np.ascontiguousarray(xqb2), "mpT": mpT,
                        "wvx": wvx, "gc": gc, "gp": gp, "eye2": eye2})
    return in_maps


def run_on_cores(in_maps, **kw):
    nc = build_program()
    return run_bass_kernel_spmd(nc, in_maps, core_ids=list(range(NCORES)),
                                **kw)


def kernel(**inputs):
    x = np.asarray(inputs["x"])
    in_maps = make_in_maps(
        inputs["x"], inputs["Wq"], inputs["bq"], inputs["Wk"], inputs["bk"],
        inputs["Wv"], inputs["bv"], inputs["gamma_c"], inputs["gamma_p"])
    res = run_on_cores(in_maps)
    out = np.zeros((B, C, N), np.float32)
    for core in range(NCORES):
        b, qb = divmod(core, 4)
        out[b][:, qb * NQ:(qb + 1) * NQ] = res.results[core]["out"]
    return out.reshape(x.shape).astype(x.dtype, copy=False)
